# revision 1
# baseline (speedup 1.0000x reference)
"""Trainium2 Bass kernel v2 for nn_DeformableUpdatingModel.

Math (same collapse as v1):
  out[m,o] = (1/HW) * ( sum_q wsum_m[q] * Fp[q,o] + be2[o] * s_m ) + b_dc[o]
  Fp = F^T Wc^T (combined 1x1 convs), wsum_m = scatter of bilinear weights,
  s_m = sum_q wsum_m[q].

v2 vs v1:
  - fp8e4 DoubleRow matmuls for Fp production (fk fp8 via casting DMA) and the
    final contraction (4x PE throughput each).
  - PSUM->SBUF crossing of Fp in (128,512) bank copies, fp8, split ACT/Pool.
  - Tents k-last so the subtract TensorTensor hits DVE 2x; T-side 16-wide
    skewed windows (free column offsets) as the moving operand; U-side full
    width stationary; wsum PSUM accumulates start=False onto a memset with
    u duplicated via two matmuls (bases 0/64) so scatters stay in-partition.
  - Flow transpose via constant selection matrices folded with 0.0625.
"""
import sys
if '/opt/trn_rl_repo' not in sys.path:
    sys.path.insert(0, '/opt/trn_rl_repo')

import numpy as np

import concourse.bacc as bacc
import concourse.mybir as mybir
import concourse.tile as tile
from concourse.bass_utils import run_bass_kernel_spmd

F32 = mybir.dt.float32
BF16 = mybir.dt.bfloat16
FP16 = mybir.dt.float16
FP8 = mybir.dt.float8e4
I32 = mybir.dt.int32
U16 = mybir.dt.uint16
OP = mybir.AluOpType
ACT = mybir.ActivationFunctionType
DR = mybir.MatmulPerfMode.DoubleRow

B, T, GOP = 4, 16, 4
NUM_GOP = T // GOP
NFLOW = 48
C = 256
H = W = 64
HW = H * W
NCORES = 8
GOPS_PER_CORE = 2
FLOWS_PER_CORE = 6

TW = 12          # T-side skewed window width; window for k is [2k-5, 2k+7)
VOFF = [2 * k - 5 for k in range(32)]


def build_nc():
    nc = bacc.Bacc("TRN2", target_bir_lowering=False, debug=False,
                   num_devices=NCORES)

    d_if = nc.dram_tensor("ifeat", [GOPS_PER_CORE, C, HW], F32, kind="ExternalInput")
    d_pm = nc.dram_tensor("pmot", [FLOWS_PER_CORE, 2, 256, 256], F32, kind="ExternalInput")
    d_wc = nc.dram_tensor("wc", [128, 2, C], F32, kind="ExternalInput")
    d_be2 = nc.dram_tensor("be2", [3, C], F32, kind="ExternalInput")
    d_bdc3 = nc.dram_tensor("bdc3", [3, C], F32, kind="ExternalInput")
    d_out = nc.dram_tensor("out", [FLOWS_PER_CORE, C], F32, kind="ExternalOutput")

    with tile.TileContext(nc) as tc:
        with (
            tc.tile_pool(name="const", bufs=1) as cpool,
            tc.tile_pool(name="wpool", bufs=1) as wpool,
            tc.tile_pool(name="fkp", bufs=2) as fkp,
            tc.tile_pool(name="fpt", bufs=2) as fptp,
            tc.tile_pool(name="work", bufs=3) as work,
            tc.tile_pool(name="flw", bufs=1) as flw,
            tc.tile_pool(name="tt", bufs=3) as ttp,
            tc.tile_pool(name="tu", bufs=3) as tup,
            tc.tile_pool(name="ws", bufs=2) as wsp,
            tc.tile_pool(name="ps_f", bufs=2, space="PSUM") as ps_f,
            tc.tile_pool(name="ps_w", bufs=2, space="PSUM") as ps_w,
            tc.tile_pool(name="ps_q", bufs=1, space="PSUM") as ps_q,
            tc.tile_pool(name="ps_o", bufs=1, space="PSUM") as ps_o,
        ):
            # ------------- input DMAs first -------------
            pts = []
            def load_pt(fg):
                pmv = d_pm[fg:fg + 1, :, :, :].squeeze(0) \
                    .rearrange("c (i f) w -> i c f w", f=4)
                pt = flw.tile([64, 2, 2, 256], F32, tag=f"pm{fg}", name=f"pt{fg}")
                nc.sync.dma_start(pt[:], pmv[:, :, 1:3, :])
                pts.append(pt)
            for fg in range(3):
                load_pt(fg)

            wct = wpool.tile([128, 2, C], FP8)
            nc.gpsimd.dma_start(wct[:], d_wc[:])
            be2 = wpool.tile([3, C], F32)
            nc.sync.dma_start(be2[:], d_be2[:])
            bdc3 = wpool.tile([3, C], F32)
            nc.sync.dma_start(bdc3[:], d_bdc3[:])

            # ------------- constants -------------
            ones16 = cpool.tile([128, 1], FP16)
            nc.gpsimd.memset(ones16[:], 1.0)


            # T-side skewed iota: iotw[p, j, k] = j - 7 - p//64 (k-independent)
            iotw = cpool.tile([128, TW, 32], FP16)
            nc.gpsimd.iota(iotw[:], pattern=[[1, TW], [0, 32]], base=-5,
                           channel_multiplier=0,
                           allow_small_or_imprecise_dtypes=True)
            nc.vector.tensor_scalar(iotw[64:128, :, :], iotw[64:128, :, :], 1.0,
                                    None, op0=OP.subtract)

            # U-side full-width iota: iotu[p, u, k] = u - p%64 (k-independent)
            iotu = cpool.tile([128, 64, 32], FP16)
            nc.gpsimd.iota(iotu[:], pattern=[[1, 64], [0, 32]], base=0,
                           channel_multiplier=-1,
                           allow_small_or_imprecise_dtypes=True)
            nc.vector.tensor_scalar(iotu[64:128, :, :], iotu[64:128, :, :], 64.0,
                                    None, op0=OP.add)

            # E_par[r, par, k] = 0.0625 * (r == 2k + par)  (transpose+downscale)
            eseli = cpool.tile([64, 2, 32], I32)
            nc.gpsimd.iota(eseli[:], pattern=[[-1, 2], [-2, 32]], base=0,
                           channel_multiplier=1)
            esel = cpool.tile([64, 2, 32], BF16)
            nc.gpsimd.tensor_scalar(esel[:], eseli[:], 0, 0.0625,
                                    op0=OP.is_equal, op1=OP.mult)

            # fk casting DMAs (after iotas on the gpsimd queue)
            fk = [fkp.tile([128, 2, HW], FP8, tag="fk", name=f"fk{g}")
                  for g in range(2)]
            nc.gpsimd.dma_start(
                fk[0][:, :, 0:2048],
                d_if[0].rearrange("(kc p) q -> p kc q", p=128)[:, :, 0:2048])
            nc.gpsimd.dma_start(
                fk[0][:, :, 2048:4096],
                d_if[0].rearrange("(kc p) q -> p kc q", p=128)[:, :, 2048:4096])
            for fg in range(3, 6):
                load_pt(fg)
            nc.gpsimd.dma_start(
                fk[1][:, :, 0:2048],
                d_if[1].rearrange("(kc p) q -> p kc q", p=128)[:, :, 0:2048])
            nc.gpsimd.dma_start(
                fk[1][:, :, 2048:4096],
                d_if[1].rearrange("(kc p) q -> p kc q", p=128)[:, :, 2048:4096])

            # ------------- all six flows: downsample + transpose + yx -------------
            yxs = []
            for fg in range(6):
                pt = pts[fg]
                tA = work.tile([64, 2, 2, 64], F32, tag="tA", name=f"tA{fg}")
                nc.vector.tensor_tensor(out=tA[:], in0=pt[:, :, :, 1:254:4],
                                        in1=pt[:, :, :, 2:255:4], op=OP.add)
                ds2 = work.tile([64, 2, 64], BF16, tag="ds2", name=f"ds2{fg}")
                nc.vector.tensor_tensor(out=ds2[:], in0=tA[:, :, 0, :],
                                        in1=tA[:, :, 1, :], op=OP.add)
                # pq[64*par + s, 32*c + k] = 0.0625 * ds2[2k+par, c, s]
                pq = ps_q.tile([128, 64], F32, tag="pq", name=f"pq{fg}")
                for comp in range(2):
                    for par in range(2):
                        nc.tensor.matmul(
                            pq[64 * par:64 * (par + 1), 32 * comp:32 * (comp + 1)],
                            ds2[:, comp, :], esel[:, par, :],
                            start=True, stop=True)
                yx = flw.tile([128, 2, 32], FP16, tag=f"yx{fg}", name=f"yx{fg}")
                nc.scalar.copy(yx[:], pq[:].rearrange("p (a b) -> p a b", a=2))
                yxs.append(yx)

            # ------------- per gop -------------
            # crossing engine per (g, 4-chunk bank-pair): A=ACT V=DVE
            # (GPSIMD cannot access PSUM on hardware)
            XENG = {0: "AAVAAVAA", 1: "AAAAAAAA"}

            def produce_banks(g, fpt, b0, b1):
                for b in range(b0, b1):
                    psf = ps_f.tile([128, 1024], F32, tag="psf",
                                    name=f"psf{g}_{b}")
                    for h in range(4):
                        c = 4 * b + h
                        nc.tensor.matmul(
                            psf[:, 256 * h:256 * (h + 1)],
                            fk[g][:, :, 128 * c:128 * (c + 1)],
                            wct[:], start=True, stop=True, perf_mode=DR)
                    dst = fpt[:, 4 * b:4 * b + 4, :]
                    src = psf[:].rearrange("p (a b) -> p a b", a=4)
                    if XENG[g][b] == "A":
                        nc.scalar.copy(dst, src)
                    else:
                        nc.vector.tensor_copy(dst, src)

            fpts = []
            wsumts = []
            for g in range(GOPS_PER_CORE):
                fpt = fptp.tile([128, 32, C], FP16, tag="fpt", name=f"fpt{g}")
                wsumt = wsp.tile([128, 32, 3], FP16, tag="wsumt", name=f"ws{g}")
                fpts.append(fpt)
                wsumts.append(wsumt)

                for mm in range(3):
                    fg = 3 * g + mm
                    yx = yxs[fg]
                    # T tents (skewed, negated): tT = min(|iotw - dy|, 1) - 1
                    teng = nc.gpsimd if fg != 0 else nc.vector
                    dT = ttp.tile([128, TW, 32], FP16, tag="dt", name=f"dt{fg}")
                    teng.tensor_tensor(
                        out=dT[:], in0=iotw[:],
                        in1=yx[:, 0:1, :].broadcast_to([128, TW, 32]),
                        op=OP.subtract)
                    nc.vector.tensor_scalar(dT[:].bitcast(U16), dT[:].bitcast(U16),
                                            0x7FFF, None, op0=OP.bitwise_and)
                    tT = ttp.tile([128, TW, 32], FP16, tag="tt", name=f"tt{fg}")
                    nc.vector.tensor_scalar(tT[:], dT[:], 1.0, 1.0,
                                            op0=OP.min, op1=OP.subtract)
                    # U tents: mU = min(|iotu - dx|, 1), 2 fused ops.
                    # (and + integer-min works: positive fp16 bit patterns
                    # order like unsigned ints; 0x3C00 is fp16 1.0)
                    tU = tup.tile([128, 64, 32], FP16, tag="tu", name=f"tu{fg}")
                    ueng = nc.gpsimd if fg >= 4 else nc.vector
                    ueng.tensor_tensor(
                        out=tU[:], in0=iotu[:],
                        in1=yx[:, 1:2, :].broadcast_to([128, 64, 32]),
                        op=OP.subtract)
                    nc.vector.tensor_scalar(tU[:].bitcast(U16), tU[:].bitcast(U16),
                                            0x7FFF, None, op0=OP.bitwise_and)
                    nc.vector.tensor_scalar(tU[:], tU[:], 1.0, 1.0,
                                            op0=OP.min, op1=OP.subtract)

                    # wsum: pw[u(+64dup), v] accumulated over k windows
                    pw = ps_w.tile([128, 64], F32, tag="pw", name=f"pw{fg}")
                    nc.vector.memset(pw[:], 0.0)
                    for k in range(32):
                        j0 = max(0, -VOFF[k])
                        j1 = min(TW, 64 - VOFF[k])
                        va, vb = VOFF[k] + j0, VOFF[k] + j1
                        last = (k == 31)
                        nc.tensor.matmul(pw[0:64, va:vb], tU[:, :, k],
                                         tT[:, j0:j1, k], start=False,
                                         stop=False, skip_group_check=True)
                        nc.tensor.matmul(pw[64:128, va:vb], tU[:, :, k],
                                         tT[:, j0:j1, k], start=False,
                                         stop=last, skip_group_check=True)
                    # scatter: wsumt[p, c, m] = wsum[u=p%64, v=2c+p//64]
                    seng = nc.vector if g == 1 else nc.scalar
                    if g == 1:
                        nc.vector.tensor_copy(wsumt[0:64, :, mm:mm + 1],
                                              pw[0:64, 0:64:2].unsqueeze(2))
                        nc.vector.tensor_copy(wsumt[64:128, :, mm:mm + 1],
                                              pw[64:128, 1:64:2].unsqueeze(2))
                    else:
                        nc.scalar.copy(wsumt[0:64, :, mm:mm + 1],
                                       pw[0:64, 0:64:2].unsqueeze(2))
                        nc.scalar.copy(wsumt[64:128, :, mm:mm + 1],
                                       pw[64:128, 1:64:2].unsqueeze(2))

                produce_banks(g, fpt, 0, 8)

                # --- contraction ---
                po = ps_o.tile([3, 272], F32, tag="po", name=f"po{g}")
                for c in range(32):
                    nc.tensor.matmul(po[:, 0:256], wsumt[:, c, :],
                                     fpt[:, c, :],
                                     start=(c == 0), stop=(c == 31),
                                     skip_group_check=True)
                    nc.tensor.matmul(po[:, 256:257], wsumt[:, c, :],
                                     ones16[:], start=(c == 0), stop=(c == 31),
                                     skip_group_check=True)

                aux = work.tile([3, C], F32, tag="aux", name=f"aux{g}")
                nc.vector.tensor_scalar(aux[:], be2[:], po[:, 256:257], None,
                                        op0=OP.mult)
                nc.vector.scalar_tensor_tensor(aux[:], in0=po[:, 0:256],
                                               scalar=1.0 / 256.0, in1=aux[:],
                                               op0=OP.mult, op1=OP.add)
                osb = work.tile([3, C], F32, tag="osb", name=f"osb{g}")
                nc.vector.scalar_tensor_tensor(osb[:], in0=aux[:],
                                               scalar=1.0 / HW, in1=bdc3[:],
                                               op0=OP.mult, op1=OP.add)
                nc.sync.dma_start(d_out[3 * g:3 * (g + 1), :], osb[:])

    nc.compile()
    return nc


_NC_CACHE = {}


def _get_nc():
    if "nc" not in _NC_CACHE:
        _NC_CACHE["nc"] = build_nc()
    return _NC_CACHE["nc"]


def make_in_maps(i_features, p_motions, W_emb, b_emb, W_dc, b_dc):
    i_features = np.ascontiguousarray(i_features, np.float32).reshape(16, C, HW)
    pm = np.ascontiguousarray(p_motions, np.float32).reshape(NFLOW, 2, 256, 256)
    wc = (np.asarray(W_dc, np.float64) @ np.asarray(W_emb, np.float64)) * 256.0
    # wct[p, kc, o] = 256 * Wc[o, 128kc + p]
    wcT = np.ascontiguousarray(
        wc.T.reshape(2, 128, C).transpose(1, 0, 2).astype(np.float32))
    be2 = (np.asarray(W_dc, np.float64) @ np.asarray(b_dc, np.float64) * 0
           + np.asarray(W_dc, np.float64) @ np.asarray(b_emb, np.float64))
    be2 = np.ascontiguousarray(
        np.repeat(be2.astype(np.float32)[None, :], 3, axis=0))
    bdc3 = np.ascontiguousarray(
        np.repeat(np.asarray(b_dc, np.float32)[None, :], 3, axis=0))
    in_maps = []
    for c in range(NCORES):
        in_maps.append({
            "ifeat": np.ascontiguousarray(i_features[2 * c:2 * c + 2]),
            "pmot": np.ascontiguousarray(pm[6 * c:6 * c + 6]),
            "wc": wcT,
            "be2": be2,
            "bdc3": bdc3,
        })
    return in_maps


def kernel(imgs, i_features, p_motions, W_emb, b_emb, W_dc, b_dc, _trace=False):
    nc = _get_nc()
    in_maps = make_in_maps(np.asarray(i_features), np.asarray(p_motions),
                           np.asarray(W_emb), np.asarray(b_emb),
                           np.asarray(W_dc), np.asarray(b_dc))
    res = run_bass_kernel_spmd(nc, in_maps, core_ids=list(range(NCORES)),
                               trace=_trace)
    out = np.concatenate([np.asarray(r["out"]) for r in res.results], axis=0)
    out = out.reshape(B, NUM_GOP, GOP - 1, C)
    if _trace:
        return out, res
    return out



# revision 12
# speedup vs baseline: 1.1156x; 1.1156x over previous
"""Trainium2 Bass kernel v3 for nn_DeformableUpdatingModel.

Math (same collapse as v2):
  out[m,o] = (1/HW) * ( sum_q W_m[q] * (Wc @ F)[o,q] + be2[o] * s_m ) + b_dc[o]
  W_m = bilinear scatter ("splat") of flow weights, s_m = sum_q W_m[q].

v3 restructure vs v2:
  - NO Fp production / PSUM->SBUF crossing.  Host ships the features
    q-TRANSPOSED in fp8 (fkT[p,ch,c] = F[c, 128*ch+p]); PE contracts
    fwT[c,m] = sum_q fkT[q,c] * wsum[q,m] directly (mixed fp8 x fp16
    matmul, out free=3 so PE engine time ~0), then po = fwT^T @ wct16.
  - All casts/transposes/iotas precomputed on host; every DMA is HWDGE
    (no Pool SWDGE descriptor time, no on-device iota).
  - Positive tents t = 1 - min(|d|,1) via TT(sub) + TS(and,min as u16)
    + TS(sub 1, mult -1); per-op engine assignment balances DVE/ACT/Pool
    (ACT variant: Abs then Relu(-x+1)).
  - Flow columns pre-sliced on host (halves pt DMA bytes).
  - wsum matmuls use a dup-broadcast stationary (one matmul per k).
"""
import sys
if '/opt/trn_rl_repo' not in sys.path:
    sys.path.insert(0, '/opt/trn_rl_repo')

import numpy as np

import concourse.bacc as bacc
import concourse.mybir as mybir
import concourse.tile as tile
from concourse.bass_utils import run_bass_kernel_spmd

F32 = mybir.dt.float32
BF16 = mybir.dt.bfloat16
FP16 = mybir.dt.float16
FP8 = mybir.dt.float8e4
U16 = mybir.dt.uint16
OP = mybir.AluOpType
ACT = mybir.ActivationFunctionType

B, T, GOP = 4, 16, 4
NUM_GOP = T // GOP
C = 256
H = W = 64
HW = H * W
NCORES = 8
GOPS_PER_CORE = 2
FLOWS_PER_CORE = 6

TW = 12          # T-side window width; window for k is [2k-5, 2k+7)
VOFF = [2 * k - 5 for k in range(32)]

# per-flow engine assignment for the tent ops (tunable):
#   usub/tsub: 'V' (DVE) or 'P' (Pool)   -- the big TensorTensor subtract
#   upost/tpost: 'V' (DVE TS pair) or 'A' (ACT Abs+Relu pair)
USUB = ['V', 'V', 'P', 'V', 'V', 'P']
UPOST = ['A', 'V', 'V', 'A', 'V', 'V']
TSUB = ['V', 'V', 'V', 'P', 'P', 'V']
TPOST = ['A', 'V', 'V', 'A', 'V', 'V']
PREP1 = ['V', 'V', 'V', 'V', 'V', 'V']


def build_nc():
    nc = bacc.Bacc("TRN2", target_bir_lowering=False, debug=False,
                   num_devices=NCORES)

    d_fkt = nc.dram_tensor("fkt", [GOPS_PER_CORE, 128, 32, C], FP8,
                           kind="ExternalInput")
    d_pt = nc.dram_tensor("ptd", [FLOWS_PER_CORE, 64, 2, 2, 64, 2], F32,
                          kind="ExternalInput")
    d_iotau = nc.dram_tensor("iotau", [128, 64 + TW, 32], FP16,
                             kind="ExternalInput")
    d_esel = nc.dram_tensor("esel", [64, 2, 32], BF16, kind="ExternalInput")
    d_wct = nc.dram_tensor("wct16", [128, 2, C], FP16, kind="ExternalInput")
    d_be2 = nc.dram_tensor("be2p", [3, C], F32, kind="ExternalInput")
    d_bdc3 = nc.dram_tensor("bdc3", [3, C], F32, kind="ExternalInput")
    d_out = nc.dram_tensor("out", [FLOWS_PER_CORE, C], F32,
                           kind="ExternalOutput")

    with tile.TileContext(nc) as tc:
        with (
            tc.tile_pool(name="const", bufs=1) as cpool,
            tc.tile_pool(name="wpool", bufs=1) as wpool,
            tc.tile_pool(name="fkp", bufs=1) as fkp,
            tc.tile_pool(name="flw", bufs=1) as flw,
            tc.tile_pool(name="work", bufs=2) as work,
            tc.tile_pool(name="du", bufs=3) as dup,
            tc.tile_pool(name="tu", bufs=3) as tup,
            tc.tile_pool(name="tt", bufs=3) as ttp,
            tc.tile_pool(name="ws", bufs=2) as wsp,
            tc.tile_pool(name="fin", bufs=2) as finp,
            tc.tile_pool(name="ps_q", bufs=2, space="PSUM") as ps_q,
            tc.tile_pool(name="ps_w", bufs=2, space="PSUM") as ps_w,
            tc.tile_pool(name="ps_f", bufs=2, space="PSUM") as ps_f,
            tc.tile_pool(name="ps_o", bufs=2, space="PSUM") as ps_o,
        ):
            # ---------------- input DMAs (all HWDGE, on sync queue) --------
            iotau = cpool.tile([128, 64 + TW, 32], FP16)
            nc.sync.dma_start(iotau[:], d_iotau[:])
            esel = cpool.tile([64, 2, 32], BF16)
            nc.sync.dma_start(esel[:], d_esel[:])

            pts = []
            for f in range(FLOWS_PER_CORE):
                pt = flw.tile([64, 2, 2, 64, 2], F32, tag=f"pm{f}",
                              name=f"pt{f}")
                pts.append(pt)

            fkts = []
            for g in range(GOPS_PER_CORE):
                fkt = fkp.tile([128, 32, C], FP8, tag=f"fk{g}", name=f"fk{g}")
                fkts.append(fkt)

            # order: pt0,pt1,pt2, fkt0, pt3,pt4,pt5, fkt1, weights
            for f in range(3):
                nc.sync.dma_start(pts[f][:], d_pt[f])
            nc.sync.dma_start(fkts[0][:], d_fkt[0])
            for f in range(3, 6):
                nc.sync.dma_start(pts[f][:], d_pt[f])
            nc.sync.dma_start(fkts[1][:], d_fkt[1])

            wct = wpool.tile([128, 2, C], FP16)
            nc.sync.dma_start(wct[:], d_wct[:])
            be2 = wpool.tile([3, C], F32)
            nc.sync.dma_start(be2[:], d_be2[:])
            bdc3 = wpool.tile([3, C], F32)
            nc.sync.dma_start(bdc3[:], d_bdc3[:])

            iotu = iotau[:, 0:64, :]
            iotw = iotau[:, 64:64 + TW, :]

            ones32 = cpool.tile([128, 1], F32)
            nc.vector.memset(ones32[:], 1.0)

            def eng(code):
                return {'V': nc.vector, 'P': nc.gpsimd, 'A': nc.scalar}[code]

            # ---------------- per-flow: prep + tents -----------------------
            yxs, tus, tts = [], [], []

            def flow_front(f):
                """downsample + transpose + yx for flow f"""
                pt = pts[f]
                t1 = work.tile([64, 2, 2, 64], F32, tag="t1", name=f"t1{f}")
                e1 = eng(PREP1[f])
                e1.tensor_tensor(out=t1[:], in0=pt[:, :, :, :, 0],
                                 in1=pt[:, :, :, :, 1], op=OP.add)
                ds2 = work.tile([64, 2, 64], BF16, tag="ds2", name=f"ds2{f}")
                e1.tensor_tensor(out=ds2[:], in0=t1[:, :, 0, :],
                                 in1=t1[:, :, 1, :], op=OP.add)
                # pq[64*par + s, 32*c + k] = 0.0625 * ds2[2k+par, c, s]
                pq = ps_q.tile([128, 64], F32, tag="pq", name=f"pq{f}")
                for comp in range(2):
                    for par in range(2):
                        nc.tensor.matmul(
                            pq[64 * par:64 * (par + 1),
                               32 * comp:32 * (comp + 1)],
                            ds2[:, comp, :], esel[:, par, :],
                            start=True, stop=True)
                yx = flw.tile([128, 2, 32], FP16, tag=f"yx{f}", name=f"yx{f}")
                nc.vector.tensor_copy(yx[:],
                                      pq[:].rearrange("p (a b) -> p a b", a=2))
                yxs.append(yx)

            def flow_tents(f):
                yx = yxs[f]
                # ---- U side: tU = 1 - min(|iotu - dx|, 1)  [128, 64, 32]
                dU = dup.tile([128, 64, 32], FP16, tag="du", name=f"du{f}")
                eng(USUB[f]).tensor_tensor(
                    out=dU[:], in0=iotu,
                    in1=yx[:, 1:2, :].broadcast_to([128, 64, 32]),
                    op=OP.subtract)
                # sign: ACT flows build +tent, DVE flows -tent (consistent
                # per flow because UPOST[f] == TPOST[f], so products match)
                tU = tup.tile([128, 64, 32], FP16, tag="tu", name=f"tu{f}")
                if UPOST[f] == 'A':
                    nc.scalar.activation(dU[:], dU[:], ACT.Abs)
                    nc.scalar.activation(tU[:], dU[:], ACT.Relu,
                                         bias=1.0, scale=-1.0)
                else:
                    e = eng(UPOST[f])
                    e.tensor_scalar(dU[:].bitcast(U16), dU[:].bitcast(U16),
                                    0x7FFF, None, op0=OP.bitwise_and)
                    e.tensor_scalar(tU[:], dU[:], 1.0, 1.0,
                                    op0=OP.min, op1=OP.subtract)
                # ---- T side: tT = 1 - min(|iotw - dy|, 1)  [128, TW, 32]
                dT = ttp.tile([128, TW, 32], FP16, tag="dt", name=f"dt{f}")
                eng(TSUB[f]).tensor_tensor(
                    out=dT[:], in0=iotw,
                    in1=yx[:, 0:1, :].broadcast_to([128, TW, 32]),
                    op=OP.subtract)
                tT = ttp.tile([128, TW, 32], FP16, tag="tt", name=f"tt{f}")
                if TPOST[f] == 'A':
                    nc.scalar.activation(dT[:], dT[:], ACT.Abs)
                    nc.scalar.activation(tT[:], dT[:], ACT.Relu,
                                         bias=1.0, scale=-1.0)
                else:
                    e = eng(TPOST[f])
                    e.tensor_scalar(dT[:].bitcast(U16), dT[:].bitcast(U16),
                                    0x7FFF, None, op0=OP.bitwise_and)
                    e.tensor_scalar(tT[:], dT[:], 1.0, 1.0,
                                    op0=OP.min, op1=OP.subtract)
                tus.append(tU)
                tts.append(tT)

            def flow_wsum(f, pw3):
                """accumulate flow f's splat into pw3[:, m, :] (m = f % 3)"""
                mm = f % 3
                tU, tT = tus[f], tts[f]
                for k in range(32):
                    j0 = max(0, -VOFF[k])
                    j1 = min(TW, 64 - VOFF[k])
                    va, vb = VOFF[k] + j0, VOFF[k] + j1
                    last = (k == 31)
                    nc.tensor.matmul(pw3[0:64, mm, va:vb], tU[:, :, k],
                                     tT[:, j0:j1, k], start=False,
                                     stop=False, skip_group_check=True)
                    nc.tensor.matmul(pw3[64:128, mm, va:vb], tU[:, :, k],
                                     tT[:, j0:j1, k], start=False,
                                     stop=last, skip_group_check=True)

            # ---------------- per gop ------------------------------------
            for g in range(GOPS_PER_CORE):
                pw3 = ps_w.tile([128, 3, 64], F32, tag="pw", name=f"pw{g}")
                nc.scalar.memzero(pw3[:])
                for mm in range(3):
                    f = 3 * g + mm
                    flow_front(f)
                    flow_tents(f)
                    flow_wsum(f, pw3)

                # scatter: wsumt[p, ch, m] = W_m[q = 128*ch + p]
                wsumt = wsp.tile([128, 32, 3], FP16, tag="ws", name=f"ws{g}")
                nc.scalar.copy(wsumt[0:64].rearrange("p c m -> p m c"),
                               pw3[0:64, :, 0:64:2])
                nc.scalar.copy(wsumt[64:128].rearrange("p c m -> p m c"),
                               pw3[64:128, :, 1:64:2])

                # s_m = sum_q W_m[q]
                # NOTE: pw3 rows are duplicated across partition halves, so
                # sum only rows 0:64 for s_m.
                smsb = finp.tile([64, 3], F32, tag="sm", name=f"sm{g}")
                nc.vector.tensor_reduce(smsb[:], pw3[0:64, :, :],
                                        axis=mybir.AxisListType.X, op=OP.add)
                po = ps_o.tile([3, 272], F32, tag="po", name=f"po{g}")
                nc.tensor.matmul(po[:, 256:257], smsb[:], ones32[0:64, :],
                                 start=True, stop=True, skip_group_check=True)

                # early bias assembly (off the critical path)
                tpre = finp.tile([3, C], F32, tag="tp", name=f"tp{g}")
                nc.vector.tensor_scalar(tpre[:], be2[:], po[:, 256:257], None,
                                        op0=OP.mult)
                nc.vector.tensor_tensor(out=tpre[:], in0=tpre[:], in1=bdc3[:],
                                        op=OP.add)

                # fwT[c, m] = sum_q fkT[q, c] * W_m[q]   (fp8 x fp16)
                fwt = ps_f.tile([128, 2, 4], F32, tag="fw", name=f"fw{g}")
                fkt = fkts[g]
                for ch in range(32):
                    st = (ch == 0)
                    sp = (ch == 31)
                    nc.tensor.matmul(fwt[:, 0, 0:3], fkt[:, ch, 0:128],
                                     wsumt[:, ch, :], start=st, stop=sp,
                                     skip_group_check=True)
                    nc.tensor.matmul(fwt[:, 1, 0:3], fkt[:, ch, 128:256],
                                     wsumt[:, ch, :], start=st, stop=sp,
                                     skip_group_check=True)
                fwts = finp.tile([128, 2, 3], FP16, tag="fws", name=f"fws{g}")
                nc.vector.tensor_copy(fwts[:, 0, :], fwt[:, 0, 0:3])
                nc.vector.tensor_copy(fwts[:, 1, :], fwt[:, 1, 0:3])

                # po[m, o] = sum_c fwT[c, m] * wc[o, c] / HW
                nc.tensor.matmul(po[:, 0:256], fwts[:, 0, :], wct[:, 0, :],
                                 start=True, stop=False,
                                 skip_group_check=True)
                nc.tensor.matmul(po[:, 0:256], fwts[:, 1, :], wct[:, 1, :],
                                 start=False, stop=True,
                                 skip_group_check=True)

                osb = finp.tile([3, C], F32, tag="osb", name=f"osb{g}")
                nc.vector.scalar_tensor_tensor(osb[:], in0=po[:, 0:256],
                                               scalar=1.0, in1=tpre[:],
                                               op0=OP.mult, op1=OP.add)
                nc.sync.dma_start(d_out[3 * g:3 * (g + 1), :], osb[:])

    nc.compile()
    return nc


_NC_CACHE = {}


def _get_nc():
    if "nc" not in _NC_CACHE:
        _NC_CACHE["nc"] = build_nc()
    return _NC_CACHE["nc"]


def make_in_maps(i_features, p_motions, W_emb, b_emb, W_dc, b_dc):
    i_features = np.ascontiguousarray(i_features, np.float32).reshape(16, C, HW)
    pm = np.ascontiguousarray(p_motions, np.float32).reshape(48, 2, 256, 256)

    np_fp8 = mybir.dt.np(FP8)
    np_bf16 = mybir.dt.np(BF16)

    # fkT[g, p, ch, c] = fp8(F[c, q]),  q = 128*ch + p
    f8 = i_features.astype(np_fp8)                 # [16, C, HW]
    fkt_all = np.ascontiguousarray(
        f8.transpose(0, 2, 1).reshape(16, 32, 128, C).transpose(0, 2, 1, 3))

    # ptd[f, i, comp, frow, j, fcol] = pm[f, comp, 4i+1+frow, 4j+1+fcol]
    pmv = pm.reshape(48, 2, 64, 4, 64, 4)[:, :, :, 1:3, :, 1:3]
    ptd_all = np.ascontiguousarray(pmv.transpose(0, 2, 1, 3, 4, 5))

    # iotau: [:, 0:64, :] = u - p%64 ; [:, 64:76, :] = j - 5 - p//64
    p_idx = np.arange(128)
    iotu = (np.arange(64)[None, :, None] - (p_idx % 64)[:, None, None]
            ).astype(np.float16) + np.zeros((1, 1, 32), np.float16)
    iotw = (np.arange(TW)[None, :, None] - 5 - (p_idx // 64)[:, None, None]
            ).astype(np.float16) + np.zeros((1, 1, 32), np.float16)
    iotau = np.ascontiguousarray(np.concatenate([iotu, iotw], axis=1))

    # esel[r, par, k] = 0.0625 * (r == 2k + par)
    r = np.arange(64)[:, None, None]
    par = np.arange(2)[None, :, None]
    k = np.arange(32)[None, None, :]
    esel = (0.0625 * (r == 2 * k + par)).astype(np_bf16)
    esel = np.ascontiguousarray(esel)

    wc = (np.asarray(W_dc, np.float64) @ np.asarray(W_emb, np.float64)) / HW
    # wct16[p, h, o] = wc[o, 128h + p] / HW
    wct16 = np.ascontiguousarray(
        wc.T.reshape(2, 128, C).transpose(1, 0, 2).astype(np.float16))
    be2 = (np.asarray(W_dc, np.float64) @ np.asarray(b_emb, np.float64)) / HW
    be2p = np.ascontiguousarray(
        np.repeat(be2.astype(np.float32)[None, :], 3, axis=0))
    bdc3 = np.ascontiguousarray(
        np.repeat(np.asarray(b_dc, np.float32)[None, :], 3, axis=0))

    in_maps = []
    for cid in range(NCORES):
        in_maps.append({
            "fkt": np.ascontiguousarray(fkt_all[2 * cid:2 * cid + 2]),
            "ptd": np.ascontiguousarray(ptd_all[6 * cid:6 * cid + 6]),
            "iotau": iotau,
            "esel": esel,
            "wct16": wct16,
            "be2p": be2p,
            "bdc3": bdc3,
        })
    return in_maps


def kernel(imgs, i_features, p_motions, W_emb, b_emb, W_dc, b_dc, _trace=False):
    nc = _get_nc()
    in_maps = make_in_maps(np.asarray(i_features), np.asarray(p_motions),
                           np.asarray(W_emb), np.asarray(b_emb),
                           np.asarray(W_dc), np.asarray(b_dc))
    res = run_bass_kernel_spmd(nc, in_maps, core_ids=list(range(NCORES)),
                               trace=_trace)
    out = np.concatenate([np.asarray(r["out"]) for r in res.results], axis=0)
    out = out.reshape(B, NUM_GOP, GOP - 1, C)
    if _trace:
        return out, res
    return out


# revision 21
# speedup vs baseline: 1.1918x; 1.0683x over previous
"""Trainium2 Bass kernel v3 for nn_DeformableUpdatingModel.

Math (same collapse as v2):
  out[m,o] = (1/HW) * ( sum_q W_m[q] * (Wc @ F)[o,q] + be2[o] * s_m ) + b_dc[o]
  W_m = bilinear scatter ("splat") of flow weights, s_m = sum_q W_m[q].

v3 restructure vs v2:
  - NO Fp production / PSUM->SBUF crossing.  Host ships the features
    q-TRANSPOSED in fp8 (fkT[p,ch,c] = F[c, 128*ch+p]); PE contracts
    fwT[c,m] = sum_q fkT[q,c] * wsum[q,m] directly (mixed fp8 x fp16
    matmul, out free=3 so PE engine time ~0), then po = fwT^T @ wct16.
  - All casts/transposes/iotas precomputed on host; every DMA is HWDGE
    (no Pool SWDGE descriptor time, no on-device iota).
  - Positive tents t = 1 - min(|d|,1) via TT(sub) + TS(and,min as u16)
    + TS(sub 1, mult -1); per-op engine assignment balances DVE/ACT/Pool
    (ACT variant: Abs then Relu(-x+1)).
  - Flow columns pre-sliced on host (halves pt DMA bytes).
  - wsum matmuls use a dup-broadcast stationary (one matmul per k).
"""
import sys
if '/opt/trn_rl_repo' not in sys.path:
    sys.path.insert(0, '/opt/trn_rl_repo')

import numpy as np

import concourse.bacc as bacc
import concourse.mybir as mybir
import concourse.tile as tile
from concourse.bass_utils import run_bass_kernel_spmd

F32 = mybir.dt.float32
BF16 = mybir.dt.bfloat16
FP16 = mybir.dt.float16
FP8 = mybir.dt.float8e4
U16 = mybir.dt.uint16
OP = mybir.AluOpType
ACT = mybir.ActivationFunctionType

B, T, GOP = 4, 16, 4
NUM_GOP = T // GOP
C = 256
H = W = 64
HW = H * W
NCORES = 8
GOPS_PER_CORE = 2
FLOWS_PER_CORE = 6

TW = 12          # T-side window width; window for k is [2k-5, 2k+7)
VOFF = [2 * k - 5 for k in range(32)]

# per-flow engine assignment for the tent ops (tunable):
#   USUB_SPLIT[f] = k: Pool computes tU[:, :, k:32], DVE k0:k (None = all DVE)
#   UPOST: 'V' (DVE pair, negative tent) or 'A' (ACT Abs+Relu, positive tent)
#   TSUB: 'V' or 'P'; TPOST sign always matches UPOST of the same flow
USUB_SPLIT = {2: 6, 3: 6}
UPOST = ['A', 'V', 'V', 'V', 'A', 'V']
TSUB = ['V', 'V', 'P', 'P', 'V', 'V']


def build_nc():
    nc = bacc.Bacc("TRN2", target_bir_lowering=False, debug=False,
                   num_devices=NCORES)

    d_fkt = nc.dram_tensor("fkt", [GOPS_PER_CORE, 128, 32, C], FP8,
                           kind="ExternalInput")
    d_pt = nc.dram_tensor("ptd", [FLOWS_PER_CORE, 64, 2, 2, 64, 2], FP16,
                          kind="ExternalInput")
    d_iotau = nc.dram_tensor("iotau", [128, 64 + TW, 32], FP16,
                             kind="ExternalInput")
    d_esel = nc.dram_tensor("esel", [64, 2, 32], FP16, kind="ExternalInput")
    d_wct = nc.dram_tensor("wct16", [128, 2, C], FP16, kind="ExternalInput")
    d_be2 = nc.dram_tensor("be2p", [3, C], F32, kind="ExternalInput")
    d_bdc3 = nc.dram_tensor("bdc3", [3, C], F32, kind="ExternalInput")
    d_out = nc.dram_tensor("out", [FLOWS_PER_CORE, C], F32,
                           kind="ExternalOutput")

    with tile.TileContext(nc) as tc:
        with (
            tc.tile_pool(name="const", bufs=1) as cpool,
            tc.tile_pool(name="wpool", bufs=1) as wpool,
            tc.tile_pool(name="fkp", bufs=1) as fkp,
            tc.tile_pool(name="flw", bufs=1) as flw,
            tc.tile_pool(name="work", bufs=2) as work,
            tc.tile_pool(name="du", bufs=3) as dup,
            tc.tile_pool(name="tu", bufs=3) as tup,
            tc.tile_pool(name="tt", bufs=3) as ttp,
            tc.tile_pool(name="ws", bufs=2) as wsp,
            tc.tile_pool(name="fin", bufs=2) as finp,
            tc.tile_pool(name="ps_q", bufs=2, space="PSUM") as ps_q,
            tc.tile_pool(name="ps_w", bufs=2, space="PSUM") as ps_w,
            tc.tile_pool(name="ps_f", bufs=2, space="PSUM") as ps_f,
            tc.tile_pool(name="ps_o", bufs=2, space="PSUM") as ps_o,
        ):
            # ---------------- input DMAs (all HWDGE, on sync queue) --------
            iotau = cpool.tile([128, 64 + TW, 32], FP16)
            esel = cpool.tile([64, 2, 32], FP16)
            pts = [flw.tile([64, 2, 2, 64, 2], FP16, tag=f"pm{f}",
                            name=f"pt{f}") for f in range(FLOWS_PER_CORE)]
            fkts = [fkp.tile([128, 32, C], FP8, tag=f"fk{g}", name=f"fk{g}")
                    for g in range(GOPS_PER_CORE)]
            wct = wpool.tile([128, 2, C], FP16)
            be2 = wpool.tile([3, C], F32)
            bdc3 = wpool.tile([3, C], F32)

            # Pool-split flows (2, 3) first, iotau before the bulk loads
            nc.sync.dma_start(esel[:], d_esel[:])
            nc.sync.dma_start(pts[2][:], d_pt[2])
            nc.sync.dma_start(pts[3][:], d_pt[3])
            nc.sync.dma_start(iotau[:], d_iotau[:])
            nc.sync.dma_start(pts[0][:], d_pt[0])
            nc.sync.dma_start(pts[1][:], d_pt[1])
            nc.sync.dma_start(fkts[0][:], d_fkt[0])
            nc.sync.dma_start(pts[4][:], d_pt[4])
            nc.sync.dma_start(pts[5][:], d_pt[5])
            nc.sync.dma_start(fkts[1][:], d_fkt[1])
            nc.sync.dma_start(wct[:], d_wct[:])
            nc.sync.dma_start(be2[:], d_be2[:])
            nc.sync.dma_start(bdc3[:], d_bdc3[:])

            iotu = iotau[:, 0:64, :]
            iotw = iotau[:, 64:64 + TW, :]

            ones32 = cpool.tile([128, 1], F32)
            nc.vector.memset(ones32[:], 1.0)

            iotu = iotau[:, 0:64, :]
            iotw = iotau[:, 64:64 + TW, :]

            # --------------- tiles created up front ------------------------
            pw3s = [ps_w.tile([128, 3, 64], F32, tag="pw", name=f"pw{g}")
                    for g in range(2)]
            nc.scalar.memzero(pw3s[0][:])
            nc.scalar.memzero(pw3s[1][:])

            yxs = [None] * 6
            tus = [None] * 6
            tts = [None] * 6

            # --------------- building blocks -------------------------------
            def front_dve(f):
                """downsample on DVE + pq transpose on PE (fp16 throughout)"""
                pt = pts[f]
                t1 = work.tile([64, 2, 2, 64], FP16, tag="t1", name=f"t1{f}")
                nc.vector.tensor_tensor(out=t1[:], in0=pt[:, :, :, :, 0],
                                        in1=pt[:, :, :, :, 1], op=OP.add)
                ds2 = work.tile([64, 2, 64], FP16, tag="ds2", name=f"ds2{f}")
                nc.vector.tensor_tensor(out=ds2[:], in0=t1[:, :, 0, :],
                                        in1=t1[:, :, 1, :], op=OP.add)
                # pq[64*par + s, 32*c + k] = 0.0625 * ds2[2k+par, c, s]
                pq = ps_q.tile([128, 64], F32, tag="pq", name=f"pq{f}")
                for comp in range(2):
                    for par in range(2):
                        nc.tensor.matmul(
                            pq[64 * par:64 * (par + 1),
                               32 * comp:32 * (comp + 1)],
                            ds2[:, comp, :], esel[:, par, :],
                            start=True, stop=True)
                return pq

            def yx_act(f, pq):
                yx = flw.tile([128, 2, 32], FP16, tag=f"yx{f}", name=f"yx{f}")
                nc.scalar.copy(yx[:], pq[:].rearrange("p (a b) -> p a b", a=2))
                yxs[f] = yx

            def usub(f, e, k0, k1):
                """dU[:, :, k0:k1] = iotu - dx on engine e"""
                dU = tus[f]  # produced in place, then post ops overwrite
                e.tensor_tensor(
                    out=dU[:, :, k0:k1], in0=iotu[:, :, k0:k1],
                    in1=yxs[f][:, 1:2, k0:k1]
                        .broadcast_to([128, 64, k1 - k0]),
                    op=OP.subtract)

            def alloc_u(f):
                tus[f] = dup.tile([128, 64, 32], FP16, tag="du", name=f"du{f}")

            def upost(f):
                """dU -> tU (in a fresh tile). 'A': +tent on ACT; 'V': -tent"""
                dU = tus[f]
                tU = tup.tile([128, 64, 32], FP16, tag="tu", name=f"tu{f}")
                if UPOST[f] == 'A':
                    nc.scalar.activation(dU[:], dU[:], ACT.Abs)
                    nc.scalar.activation(tU[:], dU[:], ACT.Relu,
                                         bias=1.0, scale=-1.0)
                else:
                    nc.vector.tensor_scalar(dU[:].bitcast(U16),
                                            dU[:].bitcast(U16),
                                            0x7FFF, None, op0=OP.bitwise_and)
                    nc.vector.tensor_scalar(tU[:], dU[:], 1.0, 1.0,
                                            op0=OP.min, op1=OP.subtract)
                tus[f] = tU

            dts = [None] * 6

            def tsub_e(f, e):
                """dT = iotw - dy on engine e"""
                dT = ttp.tile([128, TW, 32], FP16, tag="dt", name=f"dt{f}")
                e.tensor_tensor(
                    out=dT[:], in0=iotw,
                    in1=yxs[f][:, 0:1, :].broadcast_to([128, TW, 32]),
                    op=OP.subtract)
                dts[f] = dT

            def tpost_e(f):
                """dT -> tT on DVE; sign matches UPOST[f]"""
                dT = dts[f]
                tT = ttp.tile([128, TW, 32], FP16, tag="tt", name=f"tt{f}")
                nc.vector.tensor_scalar(dT[:].bitcast(U16), dT[:].bitcast(U16),
                                        0x7FFF, None, op0=OP.bitwise_and)
                nc.vector.tensor_scalar(tT[:], dT[:], 1.0, 1.0,
                                        op0=OP.min, op1=OP.subtract)
                if UPOST[f] == 'A':  # flip to +tent to match the U side
                    nc.vector.tensor_scalar(tT[:], tT[:], -1.0, None,
                                            op0=OP.mult)
                tts[f] = tT

            def flow_wsum(f):
                """accumulate flow f's splat into pw3[:, m, :] (m = f % 3)"""
                mm, pw3 = f % 3, pw3s[f // 3]
                tU, tT = tus[f], tts[f]
                for k in range(32):
                    j0 = max(0, -VOFF[k])
                    j1 = min(TW, 64 - VOFF[k])
                    va, vb = VOFF[k] + j0, VOFF[k] + j1
                    last = (k == 31)
                    nc.tensor.matmul(pw3[0:64, mm, va:vb], tU[:, :, k],
                                     tT[:, j0:j1, k], start=False,
                                     stop=False, skip_group_check=True)
                    nc.tensor.matmul(pw3[64:128, mm, va:vb], tU[:, :, k],
                                     tT[:, j0:j1, k], start=False,
                                     stop=last, skip_group_check=True)

            wsumts = [None] * 2
            pos = [None] * 2
            fwtss = [None] * 2
            tpres = [None] * 2

            def scatter_g(g):
                """ACT: wsumt[p, ch, m] = W_m[q = 128*ch + p]"""
                pw3 = pw3s[g]
                wsumt = wsp.tile([128, 32, 3], FP16, tag="ws", name=f"ws{g}")
                nc.scalar.copy(wsumt[0:64].rearrange("p c m -> p m c"),
                               pw3[0:64, :, 0:64:2])
                nc.scalar.copy(wsumt[64:128].rearrange("p c m -> p m c"),
                               pw3[64:128, :, 1:64:2])
                wsumts[g] = wsumt

            def fwmm_g(g):
                """PE: fwT[c, m] = sum_q fkT[q, c] * W_m[q] (fp8 x fp16)"""
                fkt, wsumt = fkts[g], wsumts[g]
                fwt = ps_f.tile([128, 2, 4], F32, tag="fw", name=f"fw{g}")
                for ch in range(32):
                    st = (ch == 0)
                    sp = (ch == 31)
                    nc.tensor.matmul(fwt[:, 0, 0:3], fkt[:, ch, 0:128],
                                     wsumt[:, ch, :], start=st, stop=sp,
                                     skip_group_check=True)
                    nc.tensor.matmul(fwt[:, 1, 0:3], fkt[:, ch, 128:256],
                                     wsumt[:, ch, :], start=st, stop=sp,
                                     skip_group_check=True)
                fwts = finp.tile([128, 2, 3], FP16, tag="fws", name=f"fws{g}")
                nc.vector.tensor_copy(fwts[:, 0, :], fwt[:, 0, 0:3])
                nc.vector.tensor_copy(fwts[:, 1, :], fwt[:, 1, 0:3])
                fwtss[g] = fwts

            def smtr_g(g):
                """s_m = sum_q W_m[q] + early bias assembly (uncritical)"""
                wsumt = wsumts[g]
                smsb = finp.tile([128, 3], F32, tag="sm", name=f"sm{g}")
                nc.vector.tensor_reduce(smsb[:],
                                        wsumt[:].rearrange("p c m -> p m c"),
                                        axis=mybir.AxisListType.X, op=OP.add)
                po = ps_o.tile([3, 272], F32, tag="po", name=f"po{g}")
                nc.tensor.matmul(po[:, 256:257], smsb[:], ones32[:],
                                 start=True, stop=True, skip_group_check=True)
                tpre = finp.tile([3, C], F32, tag="tp", name=f"tp{g}")
                nc.vector.tensor_scalar(tpre[:], be2[:], po[:, 256:257], None,
                                        op0=OP.mult)
                nc.vector.tensor_tensor(out=tpre[:], in0=tpre[:], in1=bdc3[:],
                                        op=OP.add)
                pos[g], tpres[g] = po, tpre

            def fin_g(g):
                """PE po matmuls + DVE osb + out DMA"""
                po, fwts = pos[g], fwtss[g]
                nc.tensor.matmul(po[:, 0:256], fwts[:, 0, :], wct[:, 0, :],
                                 start=True, stop=False,
                                 skip_group_check=True)
                nc.tensor.matmul(po[:, 0:256], fwts[:, 1, :], wct[:, 1, :],
                                 start=False, stop=True,
                                 skip_group_check=True)
                osb = finp.tile([3, C], F32, tag="osb", name=f"osb{g}")
                nc.vector.scalar_tensor_tensor(osb[:], in0=po[:, 0:256],
                                               scalar=1.0, in1=tpres[g],
                                               op0=OP.mult, op1=OP.add)
                nc.sync.dma_start(d_out[3 * g:3 * (g + 1), :], osb[:])

            # --------------- ordered emission ------------------------------
            # fronts: Pool-split flows (2, 3) first
            pqs = {}
            for f in (2, 3, 0, 1, 4, 5):
                pqs[f] = front_dve(f)
                yx_act(f, pqs[f])

            # Pool queue: usub2 tail, dT2 sub, usub3 tail
            alloc_u(2)
            alloc_u(3)
            usub(2, nc.gpsimd, USUB_SPLIT[2], 32)
            tsub_e(2, nc.gpsimd)
            usub(3, nc.gpsimd, USUB_SPLIT[3], 32)

            # DVE: heads of split usubs, then f0 / f1 chains
            usub(2, nc.vector, 0, USUB_SPLIT[2])
            usub(3, nc.vector, 0, USUB_SPLIT[3])

            alloc_u(0)
            usub(0, nc.vector, 0, 32)
            upost(0)                        # ACT pair
            tsub_e(0, nc.vector)
            tpost_e(0)
            alloc_u(1)
            usub(1, nc.vector, 0, 32)
            upost(1)                        # DVE pair
            tsub_e(1, nc.vector)
            tpost_e(1)

            upost(2)                        # DVE (Pool tail ready by now)
            tpost_e(2)                      # (Pool dT2 ready by now)
            flow_wsum(2)
            flow_wsum(0)
            flow_wsum(1)
            scatter_g(0)                    # ACT (after upost0 in queue)
            fwmm_g(0)                       # PE + 2 small DVE copies

            alloc_u(4)
            usub(4, nc.vector, 0, 32)
            upost(4)                        # ACT pair
            tsub_e(4, nc.vector)
            tpost_e(4)

            upost(3)                        # DVE (Pool tail ready by now)
            tsub_e(3, nc.vector)
            tpost_e(3)
            flow_wsum(3)

            alloc_u(5)
            usub(5, nc.vector, 0, 32)
            upost(5)                        # DVE pair
            tsub_e(5, nc.vector)
            tpost_e(5)
            flow_wsum(5)
            flow_wsum(4)
            smtr_g(0)
            fin_g(0)
            scatter_g(1)
            fwmm_g(1)
            smtr_g(1)
            fin_g(1)

    nc.compile()
    return nc


_NC_CACHE = {}


def _get_nc():
    if "nc" not in _NC_CACHE:
        _NC_CACHE["nc"] = build_nc()
    return _NC_CACHE["nc"]


def make_in_maps(i_features, p_motions, W_emb, b_emb, W_dc, b_dc):
    i_features = np.ascontiguousarray(i_features, np.float32).reshape(16, C, HW)
    pm = np.ascontiguousarray(p_motions, np.float32).reshape(48, 2, 256, 256)

    np_fp8 = mybir.dt.np(FP8)
    np_bf16 = mybir.dt.np(BF16)

    # fkT[g, p, ch, c] = fp8(F[c, q]),  q = 128*ch + p
    f8 = i_features.astype(np_fp8)                 # [16, C, HW]
    fkt_all = np.ascontiguousarray(
        f8.transpose(0, 2, 1).reshape(16, 32, 128, C).transpose(0, 2, 1, 3))

    # ptd[f, i, comp, frow, j, fcol] = pm[f, comp, 4i+1+frow, 4j+1+fcol]
    pmv = pm.reshape(48, 2, 64, 4, 64, 4)[:, :, :, 1:3, :, 1:3]
    ptd_all = np.ascontiguousarray(
        pmv.transpose(0, 2, 1, 3, 4, 5).astype(np.float16))

    # iotau: [:, 0:64, :] = u - p%64 ; [:, 64:76, :] = j - 5 - p//64
    p_idx = np.arange(128)
    iotu = (np.arange(64)[None, :, None] - (p_idx % 64)[:, None, None]
            ).astype(np.float16) + np.zeros((1, 1, 32), np.float16)
    iotw = (np.arange(TW)[None, :, None] - 5 - (p_idx // 64)[:, None, None]
            ).astype(np.float16) + np.zeros((1, 1, 32), np.float16)
    iotau = np.ascontiguousarray(np.concatenate([iotu, iotw], axis=1))

    # esel[r, par, k] = 0.0625 * (r == 2k + par)
    r = np.arange(64)[:, None, None]
    par = np.arange(2)[None, :, None]
    k = np.arange(32)[None, None, :]
    esel = np.ascontiguousarray(
        (0.0625 * (r == 2 * k + par)).astype(np.float16))

    wc = (np.asarray(W_dc, np.float64) @ np.asarray(W_emb, np.float64)) / HW
    # wct16[p, h, o] = wc[o, 128h + p] / HW
    wct16 = np.ascontiguousarray(
        wc.T.reshape(2, 128, C).transpose(1, 0, 2).astype(np.float16))
    be2 = (np.asarray(W_dc, np.float64) @ np.asarray(b_emb, np.float64)) / HW
    be2p = np.ascontiguousarray(
        np.repeat(be2.astype(np.float32)[None, :], 3, axis=0))
    bdc3 = np.ascontiguousarray(
        np.repeat(np.asarray(b_dc, np.float32)[None, :], 3, axis=0))

    in_maps = []
    for cid in range(NCORES):
        in_maps.append({
            "fkt": np.ascontiguousarray(fkt_all[2 * cid:2 * cid + 2]),
            "ptd": np.ascontiguousarray(ptd_all[6 * cid:6 * cid + 6]),
            "iotau": iotau,
            "esel": esel,
            "wct16": wct16,
            "be2p": be2p,
            "bdc3": bdc3,
        })
    return in_maps


def kernel(imgs, i_features, p_motions, W_emb, b_emb, W_dc, b_dc, _trace=False):
    nc = _get_nc()
    in_maps = make_in_maps(np.asarray(i_features), np.asarray(p_motions),
                           np.asarray(W_emb), np.asarray(b_emb),
                           np.asarray(W_dc), np.asarray(b_dc))
    res = run_bass_kernel_spmd(nc, in_maps, core_ids=list(range(NCORES)),
                               trace=_trace)
    out = np.concatenate([np.asarray(r["out"]) for r in res.results], axis=0)
    out = out.reshape(B, NUM_GOP, GOP - 1, C)
    if _trace:
        return out, res
    return out


# revision 28
# speedup vs baseline: 1.2483x; 1.0473x over previous
"""Trainium2 Bass kernel v3 for nn_DeformableUpdatingModel.

Math (same collapse as v2):
  out[m,o] = (1/HW) * ( sum_q W_m[q] * (Wc @ F)[o,q] + be2[o] * s_m ) + b_dc[o]
  W_m = bilinear scatter ("splat") of flow weights, s_m = sum_q W_m[q].

v3 restructure vs v2:
  - NO Fp production / PSUM->SBUF crossing.  Host ships the features
    q-TRANSPOSED in fp8 (fkT[p,ch,c] = F[c, 128*ch+p]); PE contracts
    fwT[c,m] = sum_q fkT[q,c] * wsum[q,m] directly (mixed fp8 x fp16
    matmul, out free=3 so PE engine time ~0), then po = fwT^T @ wct16.
  - All casts/transposes/iotas precomputed on host; every DMA is HWDGE
    (no Pool SWDGE descriptor time, no on-device iota).
  - Positive tents t = 1 - min(|d|,1) via TT(sub) + TS(and,min as u16)
    + TS(sub 1, mult -1); per-op engine assignment balances DVE/ACT/Pool
    (ACT variant: Abs then Relu(-x+1)).
  - Flow columns pre-sliced on host (halves pt DMA bytes).
  - wsum matmuls use a dup-broadcast stationary (one matmul per k).
"""
import sys
if '/opt/trn_rl_repo' not in sys.path:
    sys.path.insert(0, '/opt/trn_rl_repo')

import numpy as np

import concourse.bacc as bacc
import concourse.mybir as mybir
import concourse.tile as tile
from concourse.bass_utils import run_bass_kernel_spmd

F32 = mybir.dt.float32
BF16 = mybir.dt.bfloat16
FP16 = mybir.dt.float16
FP8 = mybir.dt.float8e4
U16 = mybir.dt.uint16
OP = mybir.AluOpType
ACT = mybir.ActivationFunctionType

B, T, GOP = 4, 16, 4
NUM_GOP = T // GOP
C = 256
H = W = 64
HW = H * W
NCORES = 8
GOPS_PER_CORE = 2
FLOWS_PER_CORE = 6

TW = 12          # T-side window width; window for k is [2k-5, 2k+7)
VOFF = [2 * k - 5 for k in range(32)]

# per-flow engine assignment for the tent ops (tunable):
#   USUB_SPLIT[f] = k: Pool computes tU[:, :, k:32], DVE k0:k (None = all DVE)
#   UPOST: 'V' (DVE pair, negative tent) or 'A' (ACT Abs+Relu, positive tent)
#   TSUB: 'V' or 'P'; TPOST sign always matches UPOST of the same flow
USUB_SPLIT = {2: 8, 3: 12}
UPOST = ['A', 'V', 'V', 'V', 'A', 'V']
TSUB = ['V', 'V', 'P', 'P', 'V', 'V']


def build_nc():
    nc = bacc.Bacc("TRN2", target_bir_lowering=False, debug=False,
                   num_devices=NCORES)

    d_fkt = nc.dram_tensor("fkt", [GOPS_PER_CORE, 128, 32, C], FP8,
                           kind="ExternalInput")
    d_pt = nc.dram_tensor("ptd", [FLOWS_PER_CORE, 64, 2, 2, 2, 64], FP16,
                          kind="ExternalInput")
    d_iotau = nc.dram_tensor("iotau", [128, 64 + TW, 32], FP16,
                             kind="ExternalInput")
    d_esel = nc.dram_tensor("esel", [64, 2, 32], FP16, kind="ExternalInput")
    d_wct = nc.dram_tensor("wct16", [128, 2, C], FP16, kind="ExternalInput")
    d_be2 = nc.dram_tensor("be2p", [3, C], F32, kind="ExternalInput")
    d_bdc3 = nc.dram_tensor("bdc3", [3, C], F32, kind="ExternalInput")
    d_out = nc.dram_tensor("out", [FLOWS_PER_CORE, C], F32,
                           kind="ExternalOutput")

    with tile.TileContext(nc) as tc:
        with (
            tc.tile_pool(name="const", bufs=1) as cpool,
            tc.tile_pool(name="wpool", bufs=1) as wpool,
            tc.tile_pool(name="fkp", bufs=1) as fkp,
            tc.tile_pool(name="flw", bufs=1) as flw,
            tc.tile_pool(name="work", bufs=2) as work,
            tc.tile_pool(name="du", bufs=3) as dup,
            tc.tile_pool(name="tu", bufs=3) as tup,
            tc.tile_pool(name="tt", bufs=3) as ttp,
            tc.tile_pool(name="ws", bufs=2) as wsp,
            tc.tile_pool(name="fin", bufs=2) as finp,
            tc.tile_pool(name="ps_q", bufs=2, space="PSUM") as ps_q,
            tc.tile_pool(name="ps_w", bufs=2, space="PSUM") as ps_w,
            tc.tile_pool(name="ps_f", bufs=2, space="PSUM") as ps_f,
            tc.tile_pool(name="ps_o", bufs=2, space="PSUM") as ps_o,
        ):
            # ---------------- input DMAs (all HWDGE, on sync queue) --------
            iotau = cpool.tile([128, 64 + TW, 32], FP16)
            esel = cpool.tile([64, 2, 32], FP16)
            pts = [flw.tile([64, 2, 2, 2, 64], FP16, tag=f"pm{f}",
                            name=f"pt{f}") for f in range(FLOWS_PER_CORE)]
            fkts = [fkp.tile([128, 32, C], FP8, tag=f"fk{g}", name=f"fk{g}")
                    for g in range(GOPS_PER_CORE)]
            wct = wpool.tile([128, 2, C], FP16)
            be2 = wpool.tile([3, C], F32)
            bdc3 = wpool.tile([3, C], F32)

            # Pool-split flows (2, 3) first; fkt late (needed only by fw)
            nc.sync.dma_start(esel[:], d_esel[:])
            nc.sync.dma_start(pts[2][:], d_pt[2])
            nc.sync.dma_start(pts[3][:], d_pt[3])
            nc.sync.dma_start(pts[0][:], d_pt[0])
            nc.sync.dma_start(pts[1][:], d_pt[1])
            nc.sync.dma_start(iotau[:], d_iotau[:])
            nc.sync.dma_start(pts[4][:], d_pt[4])
            nc.sync.dma_start(pts[5][:], d_pt[5])
            nc.sync.dma_start(fkts[0][:], d_fkt[0])
            nc.sync.dma_start(fkts[1][:], d_fkt[1])
            nc.sync.dma_start(wct[:], d_wct[:])
            nc.sync.dma_start(be2[:], d_be2[:])
            nc.sync.dma_start(bdc3[:], d_bdc3[:])

            iotu = iotau[:, 0:64, :]
            iotw = iotau[:, 64:64 + TW, :]

            ones32 = cpool.tile([128, 1], F32)
            nc.vector.memset(ones32[:], 1.0)

            iotu = iotau[:, 0:64, :]
            iotw = iotau[:, 64:64 + TW, :]

            # --------------- tiles created up front ------------------------
            pw3s = [ps_w.tile([128, 3, 64], F32, tag="pw", name=f"pw{g}")
                    for g in range(2)]
            nc.scalar.memzero(pw3s[0][:])
            nc.scalar.memzero(pw3s[1][:])

            yxs = [None] * 6
            tus = [None] * 6
            tts = [None] * 6

            # --------------- building blocks -------------------------------
            def front_dve(f, e=None):
                """downsample + pq transpose on PE (fp16, packed last dim)"""
                e = e or nc.vector
                pt = pts[f]
                t1 = work.tile([64, 2, 2, 64], FP16, tag="t1", name=f"t1{f}")
                e.tensor_tensor(out=t1[:], in0=pt[:, :, :, 0, :],
                                in1=pt[:, :, :, 1, :], op=OP.add)
                ds2 = work.tile([64, 2, 64], FP16, tag="ds2", name=f"ds2{f}")
                e.tensor_tensor(out=ds2[:], in0=t1[:, :, 0, :],
                                in1=t1[:, :, 1, :], op=OP.add)
                # pq[64*par + s, 32*c + k] = 0.0625 * ds2[2k+par, c, s]
                pq = ps_q.tile([128, 64], F32, tag="pq", name=f"pq{f}")
                for comp in range(2):
                    for par in range(2):
                        nc.tensor.matmul(
                            pq[64 * par:64 * (par + 1),
                               32 * comp:32 * (comp + 1)],
                            ds2[:, comp, :], esel[:, par, :],
                            start=True, stop=True)
                return pq

            def yx_act(f, pq):
                yx = flw.tile([128, 2, 32], FP16, tag=f"yx{f}", name=f"yx{f}")
                nc.scalar.copy(yx[:], pq[:].rearrange("p (a b) -> p a b", a=2))
                yxs[f] = yx

            def usub(f, e, k0, k1):
                """dU[:, :, k0:k1] = iotu - dx on engine e"""
                dU = tus[f]  # produced in place, then post ops overwrite
                e.tensor_tensor(
                    out=dU[:, :, k0:k1], in0=iotu[:, :, k0:k1],
                    in1=yxs[f][:, 1:2, k0:k1]
                        .broadcast_to([128, 64, k1 - k0]),
                    op=OP.subtract)

            def alloc_u(f):
                tus[f] = dup.tile([128, 64, 32], FP16, tag="du", name=f"du{f}")

            def upost(f):
                """dU -> tU (in a fresh tile). 'A': +tent on ACT; 'V': -tent"""
                dU = tus[f]
                tU = tup.tile([128, 64, 32], FP16, tag="tu", name=f"tu{f}")
                if UPOST[f] == 'A':
                    nc.scalar.activation(dU[:], dU[:], ACT.Abs)
                    nc.scalar.activation(tU[:], dU[:], ACT.Relu,
                                         bias=1.0, scale=-1.0)
                else:
                    nc.vector.tensor_scalar(dU[:].bitcast(U16),
                                            dU[:].bitcast(U16),
                                            0x7FFF, None, op0=OP.bitwise_and)
                    nc.vector.tensor_scalar(tU[:], dU[:], 1.0, 1.0,
                                            op0=OP.min, op1=OP.subtract)
                tus[f] = tU

            dts = [None] * 6

            def tsub_e(f, e):
                """dT = iotw - dy on engine e"""
                dT = ttp.tile([128, TW, 32], FP16, tag="dt", name=f"dt{f}")
                e.tensor_tensor(
                    out=dT[:], in0=iotw,
                    in1=yxs[f][:, 0:1, :].broadcast_to([128, TW, 32]),
                    op=OP.subtract)
                dts[f] = dT

            def tpost_e(f):
                """dT -> tT on DVE; sign matches UPOST[f]"""
                dT = dts[f]
                tT = ttp.tile([128, TW, 32], FP16, tag="tt", name=f"tt{f}")
                nc.vector.tensor_scalar(dT[:].bitcast(U16), dT[:].bitcast(U16),
                                        0x7FFF, None, op0=OP.bitwise_and)
                nc.vector.tensor_scalar(tT[:], dT[:], 1.0, 1.0,
                                        op0=OP.min, op1=OP.subtract)
                if UPOST[f] == 'A':  # flip to +tent to match the U side
                    nc.vector.tensor_scalar(tT[:], tT[:], -1.0, None,
                                            op0=OP.mult)
                tts[f] = tT

            def flow_wsum(f):
                """accumulate flow f's splat into pw3[:, m, :] (m = f % 3)"""
                mm, pw3 = f % 3, pw3s[f // 3]
                tU, tT = tus[f], tts[f]
                for k in range(32):
                    j0 = max(0, -VOFF[k])
                    j1 = min(TW, 64 - VOFF[k])
                    va, vb = VOFF[k] + j0, VOFF[k] + j1
                    last = (k == 31)
                    nc.tensor.matmul(pw3[0:64, mm, va:vb], tU[:, :, k],
                                     tT[:, j0:j1, k], start=False,
                                     stop=False, skip_group_check=True)
                    nc.tensor.matmul(pw3[64:128, mm, va:vb], tU[:, :, k],
                                     tT[:, j0:j1, k], start=False,
                                     stop=last, skip_group_check=True)

            wsumts = [None] * 2
            pos = [None] * 2
            fwtss = [None] * 2
            tpres = [None] * 2

            def scatter_g(g):
                """ACT: wsumt[p, ch, m] = W_m[q = 128*ch + p]"""
                pw3 = pw3s[g]
                wsumt = wsp.tile([128, 32, 3], FP16, tag="ws", name=f"ws{g}")
                nc.scalar.copy(wsumt[0:64].rearrange("p c m -> p m c"),
                               pw3[0:64, :, 0:64:2])
                nc.scalar.copy(wsumt[64:128].rearrange("p c m -> p m c"),
                               pw3[64:128, :, 1:64:2])
                wsumts[g] = wsumt

            def fwmm_g(g):
                """PE: fwT[c, m] = sum_q fkT[q, c] * W_m[q] (fp8 x fp16)"""
                fkt, wsumt = fkts[g], wsumts[g]
                fwt = ps_f.tile([128, 2, 4], F32, tag="fw", name=f"fw{g}")
                for ch in range(32):
                    st = (ch == 0)
                    sp = (ch == 31)
                    nc.tensor.matmul(fwt[:, 0, 0:3], fkt[:, ch, 0:128],
                                     wsumt[:, ch, :], start=st, stop=sp,
                                     skip_group_check=True)
                    nc.tensor.matmul(fwt[:, 1, 0:3], fkt[:, ch, 128:256],
                                     wsumt[:, ch, :], start=st, stop=sp,
                                     skip_group_check=True)
                fwts = finp.tile([128, 2, 3], FP16, tag="fws", name=f"fws{g}")
                nc.vector.tensor_copy(fwts[:, 0, :], fwt[:, 0, 0:3])
                nc.vector.tensor_copy(fwts[:, 1, :], fwt[:, 1, 0:3])
                fwtss[g] = fwts

            def smtr_g(g):
                """s_m = sum_q W_m[q] + early bias assembly (uncritical)"""
                wsumt = wsumts[g]
                smsb = finp.tile([128, 3], F32, tag="sm", name=f"sm{g}")
                nc.vector.tensor_reduce(smsb[:],
                                        wsumt[:].rearrange("p c m -> p m c"),
                                        axis=mybir.AxisListType.X, op=OP.add)
                po = ps_o.tile([3, 272], F32, tag="po", name=f"po{g}")
                nc.tensor.matmul(po[:, 256:257], smsb[:], ones32[:],
                                 start=True, stop=True, skip_group_check=True)
                tpre = finp.tile([3, C], F32, tag="tp", name=f"tp{g}")
                nc.vector.tensor_scalar(tpre[:], be2[:], po[:, 256:257], None,
                                        op0=OP.mult)
                nc.vector.tensor_tensor(out=tpre[:], in0=tpre[:], in1=bdc3[:],
                                        op=OP.add)
                pos[g], tpres[g] = po, tpre

            def fin_g(g):
                """PE po matmuls + DVE osb + out DMA"""
                po, fwts = pos[g], fwtss[g]
                nc.tensor.matmul(po[:, 0:256], fwts[:, 0, :], wct[:, 0, :],
                                 start=True, stop=False,
                                 skip_group_check=True)
                nc.tensor.matmul(po[:, 0:256], fwts[:, 1, :], wct[:, 1, :],
                                 start=False, stop=True,
                                 skip_group_check=True)
                osb = finp.tile([3, C], F32, tag="osb", name=f"osb{g}")
                nc.vector.scalar_tensor_tensor(osb[:], in0=po[:, 0:256],
                                               scalar=1.0, in1=tpres[g],
                                               op0=OP.mult, op1=OP.add)
                nc.sync.dma_start(d_out[3 * g:3 * (g + 1), :], osb[:])

            # --------------- ordered emission ------------------------------
            # fronts: Pool does its own flows (2, 3) while waiting for iotau
            pqs = {}
            for f in (2, 3):
                pqs[f] = front_dve(f, nc.gpsimd)
                yx_act(f, pqs[f])
            for f in (0, 1, 4, 5):
                pqs[f] = front_dve(f)
                yx_act(f, pqs[f])

            # Pool queue: usub2 tail, dT2 sub, usub3 tail
            alloc_u(2)
            alloc_u(3)
            usub(2, nc.gpsimd, USUB_SPLIT[2], 32)
            tsub_e(2, nc.gpsimd)
            usub(3, nc.gpsimd, USUB_SPLIT[3], 32)

            # DVE: heads of split usubs, then f0 / f1 chains
            usub(2, nc.vector, 0, USUB_SPLIT[2])
            usub(3, nc.vector, 0, USUB_SPLIT[3])

            alloc_u(0)
            usub(0, nc.vector, 0, 32)
            upost(0)                        # ACT pair
            tsub_e(0, nc.vector)
            tpost_e(0)
            alloc_u(1)
            usub(1, nc.vector, 0, 32)
            upost(1)                        # DVE pair
            tsub_e(1, nc.vector)
            tpost_e(1)

            upost(2)                        # DVE (Pool tail ready by now)
            tpost_e(2)                      # (Pool dT2 ready by now)
            flow_wsum(2)
            flow_wsum(0)
            flow_wsum(1)
            scatter_g(0)                    # ACT (after upost0 in queue)
            fwmm_g(0)                       # PE + 2 small DVE copies

            alloc_u(4)
            usub(4, nc.vector, 0, 32)
            upost(4)                        # ACT pair
            tsub_e(4, nc.vector)
            tpost_e(4)

            upost(3)                        # DVE (Pool tail ready by now)
            tsub_e(3, nc.vector)
            tpost_e(3)
            flow_wsum(3)

            alloc_u(5)
            usub(5, nc.vector, 0, 32)
            upost(5)                        # DVE pair
            tsub_e(5, nc.vector)
            tpost_e(5)
            flow_wsum(5)
            flow_wsum(4)
            smtr_g(0)
            fin_g(0)
            scatter_g(1)
            fwmm_g(1)
            smtr_g(1)
            fin_g(1)

    nc.compile()
    return nc


_NC_CACHE = {}


def _get_nc():
    if "nc" not in _NC_CACHE:
        _NC_CACHE["nc"] = build_nc()
    return _NC_CACHE["nc"]


def make_in_maps(i_features, p_motions, W_emb, b_emb, W_dc, b_dc):
    i_features = np.ascontiguousarray(i_features, np.float32).reshape(16, C, HW)
    pm = np.ascontiguousarray(p_motions, np.float32).reshape(48, 2, 256, 256)

    np_fp8 = mybir.dt.np(FP8)
    np_bf16 = mybir.dt.np(BF16)

    # fkT[g, p, ch, c] = fp8(F[c, q]),  q = 128*ch + p
    f8 = i_features.astype(np_fp8)                 # [16, C, HW]
    fkt_all = np.ascontiguousarray(
        f8.transpose(0, 2, 1).reshape(16, 32, 128, C).transpose(0, 2, 1, 3))

    # ptd[f, i, comp, frow, fcol, j] = pm[f, comp, 4i+1+frow, 4j+1+fcol]
    pmv = pm.reshape(48, 2, 64, 4, 64, 4)[:, :, :, 1:3, :, 1:3]
    ptd_all = np.ascontiguousarray(
        pmv.transpose(0, 2, 1, 3, 5, 4).astype(np.float16))

    # iotau: [:, 0:64, :] = u - p%64 ; [:, 64:76, :] = j - 5 - p//64
    p_idx = np.arange(128)
    iotu = (np.arange(64)[None, :, None] - (p_idx % 64)[:, None, None]
            ).astype(np.float16) + np.zeros((1, 1, 32), np.float16)
    iotw = (np.arange(TW)[None, :, None] - 5 - (p_idx // 64)[:, None, None]
            ).astype(np.float16) + np.zeros((1, 1, 32), np.float16)
    iotau = np.ascontiguousarray(np.concatenate([iotu, iotw], axis=1))

    # esel[r, par, k] = 0.0625 * (r == 2k + par)
    r = np.arange(64)[:, None, None]
    par = np.arange(2)[None, :, None]
    k = np.arange(32)[None, None, :]
    esel = np.ascontiguousarray(
        (0.0625 * (r == 2 * k + par)).astype(np.float16))

    wc = (np.asarray(W_dc, np.float64) @ np.asarray(W_emb, np.float64)) / HW
    # wct16[p, h, o] = wc[o, 128h + p] / HW
    wct16 = np.ascontiguousarray(
        wc.T.reshape(2, 128, C).transpose(1, 0, 2).astype(np.float16))
    be2 = (np.asarray(W_dc, np.float64) @ np.asarray(b_emb, np.float64)) / HW
    be2p = np.ascontiguousarray(
        np.repeat(be2.astype(np.float32)[None, :], 3, axis=0))
    bdc3 = np.ascontiguousarray(
        np.repeat(np.asarray(b_dc, np.float32)[None, :], 3, axis=0))

    in_maps = []
    for cid in range(NCORES):
        in_maps.append({
            "fkt": np.ascontiguousarray(fkt_all[2 * cid:2 * cid + 2]),
            "ptd": np.ascontiguousarray(ptd_all[6 * cid:6 * cid + 6]),
            "iotau": iotau,
            "esel": esel,
            "wct16": wct16,
            "be2p": be2p,
            "bdc3": bdc3,
        })
    return in_maps


def kernel(imgs, i_features, p_motions, W_emb, b_emb, W_dc, b_dc, _trace=False):
    nc = _get_nc()
    in_maps = make_in_maps(np.asarray(i_features), np.asarray(p_motions),
                           np.asarray(W_emb), np.asarray(b_emb),
                           np.asarray(W_dc), np.asarray(b_dc))
    res = run_bass_kernel_spmd(nc, in_maps, core_ids=list(range(NCORES)),
                               trace=_trace)
    out = np.concatenate([np.asarray(r["out"]) for r in res.results], axis=0)
    out = out.reshape(B, NUM_GOP, GOP - 1, C)
    if _trace:
        return out, res
    return out


# revision 32
# speedup vs baseline: 1.2819x; 1.0270x over previous
"""Trainium2 Bass kernel v3 for nn_DeformableUpdatingModel.

Math (same collapse as v2):
  out[m,o] = (1/HW) * ( sum_q W_m[q] * (Wc @ F)[o,q] + be2[o] * s_m ) + b_dc[o]
  W_m = bilinear scatter ("splat") of flow weights, s_m = sum_q W_m[q].

v3 restructure vs v2:
  - NO Fp production / PSUM->SBUF crossing.  Host ships the features
    q-TRANSPOSED in fp8 (fkT[p,ch,c] = F[c, 128*ch+p]); PE contracts
    fwT[c,m] = sum_q fkT[q,c] * wsum[q,m] directly (mixed fp8 x fp16
    matmul, out free=3 so PE engine time ~0), then po = fwT^T @ wct16.
  - All casts/transposes/iotas precomputed on host; every DMA is HWDGE
    (no Pool SWDGE descriptor time, no on-device iota).
  - Positive tents t = 1 - min(|d|,1) via TT(sub) + TS(and,min as u16)
    + TS(sub 1, mult -1); per-op engine assignment balances DVE/ACT/Pool
    (ACT variant: Abs then Relu(-x+1)).
  - Flow columns pre-sliced on host (halves pt DMA bytes).
  - wsum matmuls use a dup-broadcast stationary (one matmul per k).
"""
import sys
if '/opt/trn_rl_repo' not in sys.path:
    sys.path.insert(0, '/opt/trn_rl_repo')

import numpy as np

import concourse.bacc as bacc
import concourse.mybir as mybir
import concourse.tile as tile
from concourse.bass_utils import run_bass_kernel_spmd

F32 = mybir.dt.float32
BF16 = mybir.dt.bfloat16
FP16 = mybir.dt.float16
FP8 = mybir.dt.float8e4
U16 = mybir.dt.uint16
OP = mybir.AluOpType
ACT = mybir.ActivationFunctionType

B, T, GOP = 4, 16, 4
NUM_GOP = T // GOP
C = 256
H = W = 64
HW = H * W
NCORES = 8
GOPS_PER_CORE = 2
FLOWS_PER_CORE = 6

TW = 12          # T-side window width; window for k is [2k-5, 2k+7)
VOFF = [2 * k - 5 for k in range(32)]

# per-flow engine assignment for the tent ops (tunable):
#   USUB_SPLIT[f] = k: Pool computes tU[:, :, k:32], DVE k0:k (None = all DVE)
#   UPOST: 'V' (DVE pair, negative tent) or 'A' (ACT Abs+Relu, positive tent)
#   TSUB: 'V' or 'P'; TPOST sign always matches UPOST of the same flow
USUB_SPLIT = {2: 8, 3: 12, 4: 22}
UPOST = ['A', 'V', 'V', 'V', 'A', 'V']
TSUB = ['V', 'V', 'P', 'V', 'V', 'V']


def build_nc():
    nc = bacc.Bacc("TRN2", target_bir_lowering=False, debug=False,
                   num_devices=NCORES)

    d_fkt = nc.dram_tensor("fkt", [GOPS_PER_CORE, 128, 32, C], FP8,
                           kind="ExternalInput")
    d_pt = nc.dram_tensor("ptd", [FLOWS_PER_CORE, 64, 2, 2, 2, 64], FP16,
                          kind="ExternalInput")
    d_iotau = nc.dram_tensor("iotau", [128, 64 + TW, 32], FP16,
                             kind="ExternalInput")
    d_esel = nc.dram_tensor("esel", [64, 2, 32], FP16, kind="ExternalInput")
    d_wct = nc.dram_tensor("wct16", [128, 2, C], FP16, kind="ExternalInput")
    d_be2 = nc.dram_tensor("be2p", [3, C], F32, kind="ExternalInput")
    d_bdc3 = nc.dram_tensor("bdc3", [3, C], F32, kind="ExternalInput")
    d_out = nc.dram_tensor("out", [FLOWS_PER_CORE, C], F32,
                           kind="ExternalOutput")

    with tile.TileContext(nc) as tc:
        with (
            tc.tile_pool(name="const", bufs=1) as cpool,
            tc.tile_pool(name="wpool", bufs=1) as wpool,
            tc.tile_pool(name="fkp", bufs=1) as fkp,
            tc.tile_pool(name="flw", bufs=1) as flw,
            tc.tile_pool(name="work", bufs=3) as work,
            tc.tile_pool(name="du", bufs=6) as dup,
            tc.tile_pool(name="tu", bufs=6) as tup,
            tc.tile_pool(name="tt", bufs=6) as ttp,
            tc.tile_pool(name="ws", bufs=2) as wsp,
            tc.tile_pool(name="fin", bufs=2) as finp,
            tc.tile_pool(name="ps_q", bufs=2, space="PSUM") as ps_q,
            tc.tile_pool(name="ps_w", bufs=2, space="PSUM") as ps_w,
            tc.tile_pool(name="ps_f", bufs=2, space="PSUM") as ps_f,
            tc.tile_pool(name="ps_o", bufs=2, space="PSUM") as ps_o,
        ):
            # ---------------- input DMAs (all HWDGE, on sync queue) --------
            iotau = cpool.tile([128, 64 + TW, 32], FP16)
            esel = cpool.tile([64, 2, 32], FP16)
            pts = [flw.tile([64, 2, 2, 2, 64], FP16, tag=f"pm{f}",
                            name=f"pt{f}") for f in range(FLOWS_PER_CORE)]
            fkts = [fkp.tile([128, 32, C], FP8, tag=f"fk{g}", name=f"fk{g}")
                    for g in range(GOPS_PER_CORE)]
            wct = wpool.tile([128, 2, C], FP16)
            be2 = wpool.tile([3, C], F32)
            bdc3 = wpool.tile([3, C], F32)

            # Pool-split flows (2, 3) first; fkt late (needed only by fw)
            nc.sync.dma_start(esel[:], d_esel[:])
            nc.sync.dma_start(pts[2][:], d_pt[2])
            nc.sync.dma_start(pts[3][:], d_pt[3])
            nc.sync.dma_start(iotau[:], d_iotau[:])
            nc.sync.dma_start(pts[0][:], d_pt[0])
            nc.sync.dma_start(pts[1][:], d_pt[1])
            nc.sync.dma_start(pts[4][:], d_pt[4])
            nc.sync.dma_start(pts[5][:], d_pt[5])
            nc.sync.dma_start(fkts[0][:], d_fkt[0])
            nc.sync.dma_start(fkts[1][:], d_fkt[1])
            nc.sync.dma_start(wct[:], d_wct[:])
            nc.sync.dma_start(be2[:], d_be2[:])
            nc.sync.dma_start(bdc3[:], d_bdc3[:])

            iotu = iotau[:, 0:64, :]
            iotw = iotau[:, 64:64 + TW, :]

            ones32 = cpool.tile([128, 1], F32)
            nc.vector.memset(ones32[:], 1.0)

            iotu = iotau[:, 0:64, :]
            iotw = iotau[:, 64:64 + TW, :]

            # --------------- tiles created up front ------------------------
            pw3s = [ps_w.tile([128, 3, 64], F32, tag="pw", name=f"pw{g}")
                    for g in range(2)]
            nc.scalar.memzero(pw3s[0][:])
            nc.scalar.memzero(pw3s[1][:])

            yxs = [None] * 6
            tus = [None] * 6
            tts = [None] * 6

            # --------------- building blocks -------------------------------
            def front_dve(f, e=None):
                """downsample + pq transpose on PE (fp16, packed last dim)"""
                e = e or nc.vector
                pt = pts[f]
                t1 = work.tile([64, 2, 2, 64], FP16, tag="t1", name=f"t1{f}")
                e.tensor_tensor(out=t1[:], in0=pt[:, :, :, 0, :],
                                in1=pt[:, :, :, 1, :], op=OP.add)
                ds2 = work.tile([64, 2, 64], FP16, tag="ds2", name=f"ds2{f}")
                e.tensor_tensor(out=ds2[:], in0=t1[:, :, 0, :],
                                in1=t1[:, :, 1, :], op=OP.add)
                # pq[64*par + s, 32*c + k] = 0.0625 * ds2[2k+par, c, s]
                pq = ps_q.tile([128, 64], F32, tag="pq", name=f"pq{f}")
                for comp in range(2):
                    for par in range(2):
                        nc.tensor.matmul(
                            pq[64 * par:64 * (par + 1),
                               32 * comp:32 * (comp + 1)],
                            ds2[:, comp, :], esel[:, par, :],
                            start=True, stop=True)
                return pq

            def yx_act(f, pq):
                yx = flw.tile([128, 2, 32], FP16, tag=f"yx{f}", name=f"yx{f}")
                nc.scalar.copy(yx[:], pq[:].rearrange("p (a b) -> p a b", a=2))
                yxs[f] = yx

            def usub(f, e, k0, k1):
                """dU[:, :, k0:k1] = iotu - dx on engine e"""
                dU = tus[f]  # produced in place, then post ops overwrite
                e.tensor_tensor(
                    out=dU[:, :, k0:k1], in0=iotu[:, :, k0:k1],
                    in1=yxs[f][:, 1:2, k0:k1]
                        .broadcast_to([128, 64, k1 - k0]),
                    op=OP.subtract)

            def alloc_u(f):
                tus[f] = dup.tile([128, 64, 32], FP16, tag="du", name=f"du{f}")

            def upost(f):
                """dU -> tU (in a fresh tile). 'A': +tent on ACT; 'V': -tent"""
                dU = tus[f]
                tU = tup.tile([128, 64, 32], FP16, tag="tu", name=f"tu{f}")
                if UPOST[f] == 'A':
                    nc.scalar.activation(dU[:], dU[:], ACT.Abs)
                    nc.scalar.activation(tU[:], dU[:], ACT.Relu,
                                         bias=1.0, scale=-1.0)
                else:
                    nc.vector.tensor_scalar(dU[:].bitcast(U16),
                                            dU[:].bitcast(U16),
                                            0x7FFF, None, op0=OP.bitwise_and)
                    nc.vector.tensor_scalar(tU[:], dU[:], 1.0, 1.0,
                                            op0=OP.min, op1=OP.subtract)
                tus[f] = tU

            dts = [None] * 6

            def tsub_e(f, e):
                """dT = iotw - dy on engine e"""
                dT = ttp.tile([128, TW, 32], FP16, tag="dt", name=f"dt{f}")
                e.tensor_tensor(
                    out=dT[:], in0=iotw,
                    in1=yxs[f][:, 0:1, :].broadcast_to([128, TW, 32]),
                    op=OP.subtract)
                dts[f] = dT

            def tpost_e(f):
                """dT -> tT on DVE; sign matches UPOST[f]"""
                dT = dts[f]
                tT = ttp.tile([128, TW, 32], FP16, tag="tt", name=f"tt{f}")
                nc.vector.tensor_scalar(dT[:].bitcast(U16), dT[:].bitcast(U16),
                                        0x7FFF, None, op0=OP.bitwise_and)
                nc.vector.tensor_scalar(tT[:], dT[:], 1.0, 1.0,
                                        op0=OP.min, op1=OP.subtract)
                if UPOST[f] == 'A':  # flip to +tent to match the U side
                    nc.vector.tensor_scalar(tT[:], tT[:], -1.0, None,
                                            op0=OP.mult)
                tts[f] = tT

            def flow_wsum(f):
                """accumulate flow f's splat into pw3[:, m, :] (m = f % 3)"""
                mm, pw3 = f % 3, pw3s[f // 3]
                tU, tT = tus[f], tts[f]
                for k in range(32):
                    j0 = max(0, -VOFF[k])
                    j1 = min(TW, 64 - VOFF[k])
                    va, vb = VOFF[k] + j0, VOFF[k] + j1
                    last = (k == 31)
                    nc.tensor.matmul(pw3[0:64, mm, va:vb], tU[:, :, k],
                                     tT[:, j0:j1, k], start=False,
                                     stop=False, skip_group_check=True)
                    nc.tensor.matmul(pw3[64:128, mm, va:vb], tU[:, :, k],
                                     tT[:, j0:j1, k], start=False,
                                     stop=last, skip_group_check=True)

            wsumts = [None] * 2
            pos = [None] * 2
            fwtss = [None] * 2
            tpres = [None] * 2

            def scatter_g(g):
                """ACT: wsumt[p, ch, m] = W_m[q = 128*ch + p]"""
                pw3 = pw3s[g]
                wsumt = wsp.tile([128, 32, 3], FP16, tag="ws", name=f"ws{g}")
                nc.scalar.copy(wsumt[0:64].rearrange("p c m -> p m c"),
                               pw3[0:64, :, 0:64:2])
                nc.scalar.copy(wsumt[64:128].rearrange("p c m -> p m c"),
                               pw3[64:128, :, 1:64:2])
                wsumts[g] = wsumt

            def fwmm_g(g):
                """PE: fwT[c, m] = sum_q fkT[q, c] * W_m[q] (fp8 x fp16)"""
                fkt, wsumt = fkts[g], wsumts[g]
                fwt = ps_f.tile([128, 2, 4], F32, tag="fw", name=f"fw{g}")
                for ch in range(32):
                    st = (ch == 0)
                    sp = (ch == 31)
                    nc.tensor.matmul(fwt[:, 0, 0:3], fkt[:, ch, 0:128],
                                     wsumt[:, ch, :], start=st, stop=sp,
                                     skip_group_check=True)
                    nc.tensor.matmul(fwt[:, 1, 0:3], fkt[:, ch, 128:256],
                                     wsumt[:, ch, :], start=st, stop=sp,
                                     skip_group_check=True)
                fwts = finp.tile([128, 2, 3], FP16, tag="fws", name=f"fws{g}")
                nc.vector.tensor_copy(fwts[:, 0, :], fwt[:, 0, 0:3])
                nc.vector.tensor_copy(fwts[:, 1, :], fwt[:, 1, 0:3])
                fwtss[g] = fwts

            def smtr_g(g):
                """s_m = sum_q W_m[q] + early bias assembly (uncritical)"""
                wsumt = wsumts[g]
                smsb = finp.tile([128, 3], F32, tag="sm", name=f"sm{g}")
                nc.vector.tensor_reduce(smsb[:],
                                        wsumt[:].rearrange("p c m -> p m c"),
                                        axis=mybir.AxisListType.X, op=OP.add)
                po = ps_o.tile([3, 272], F32, tag="po", name=f"po{g}")
                nc.tensor.matmul(po[:, 256:257], smsb[:], ones32[:],
                                 start=True, stop=True, skip_group_check=True)
                tpre = finp.tile([3, C], F32, tag="tp", name=f"tp{g}")
                nc.vector.tensor_scalar(tpre[:], be2[:], po[:, 256:257], None,
                                        op0=OP.mult)
                nc.vector.tensor_tensor(out=tpre[:], in0=tpre[:], in1=bdc3[:],
                                        op=OP.add)
                pos[g], tpres[g] = po, tpre

            def fin_g(g):
                """PE po matmuls + DVE osb + out DMA"""
                po, fwts = pos[g], fwtss[g]
                nc.tensor.matmul(po[:, 0:256], fwts[:, 0, :], wct[:, 0, :],
                                 start=True, stop=False,
                                 skip_group_check=True)
                nc.tensor.matmul(po[:, 0:256], fwts[:, 1, :], wct[:, 1, :],
                                 start=False, stop=True,
                                 skip_group_check=True)
                osb = finp.tile([3, C], F32, tag="osb", name=f"osb{g}")
                nc.vector.scalar_tensor_tensor(osb[:], in0=po[:, 0:256],
                                               scalar=1.0, in1=tpres[g],
                                               op0=OP.mult, op1=OP.add)
                nc.sync.dma_start(d_out[3 * g:3 * (g + 1), :], osb[:])

            # --------------- ordered emission ------------------------------
            # fronts: Pool does its own flows (2, 3) while waiting for iotau
            pqs = {}
            for f in (2, 3):
                pqs[f] = front_dve(f, nc.gpsimd)
                yx_act(f, pqs[f])
            for f in (0, 1, 4, 5):
                pqs[f] = front_dve(f)
                yx_act(f, pqs[f])

            # Pool queue: usub tails for 2, 3, 4 (+ dT2 sub)
            alloc_u(2)
            alloc_u(3)
            alloc_u(4)
            usub(2, nc.gpsimd, USUB_SPLIT[2], 32)
            tsub_e(2, nc.gpsimd)
            usub(3, nc.gpsimd, USUB_SPLIT[3], 32)
            usub(4, nc.gpsimd, USUB_SPLIT[4], 32)

            # DVE: heads of split usubs, then f0 / f1 chains
            usub(2, nc.vector, 0, USUB_SPLIT[2])
            usub(3, nc.vector, 0, USUB_SPLIT[3])

            alloc_u(0)
            usub(0, nc.vector, 0, 32)
            upost(0)                        # ACT pair
            tsub_e(0, nc.vector)
            tpost_e(0)
            alloc_u(1)
            usub(1, nc.vector, 0, 32)
            upost(1)                        # DVE pair
            tsub_e(1, nc.vector)
            tpost_e(1)

            tpost_e(2)                      # (Pool dT2 ready by now)
            upost(2)                        # DVE (Pool tail ready by now)
            flow_wsum(1)
            flow_wsum(2)
            flow_wsum(0)
            scatter_g(0)                    # ACT (after upost0 in queue)
            fwmm_g(0)                       # PE + 2 small DVE copies

            usub(4, nc.vector, 0, USUB_SPLIT[4])
            tsub_e(4, nc.vector)
            tpost_e(4)
            upost(4)                        # ACT pair (after Pool tail)

            upost(3)                        # DVE (Pool tail ready by now)
            tsub_e(3, nc.vector)
            tpost_e(3)
            flow_wsum(3)

            alloc_u(5)
            usub(5, nc.vector, 0, 32)
            upost(5)                        # DVE pair
            tsub_e(5, nc.vector)
            tpost_e(5)
            flow_wsum(5)
            flow_wsum(4)
            smtr_g(0)
            fin_g(0)
            scatter_g(1)
            fwmm_g(1)
            smtr_g(1)
            fin_g(1)

    nc.compile()
    return nc


_NC_CACHE = {}


def _get_nc():
    if "nc" not in _NC_CACHE:
        _NC_CACHE["nc"] = build_nc()
    return _NC_CACHE["nc"]


def make_in_maps(i_features, p_motions, W_emb, b_emb, W_dc, b_dc):
    i_features = np.ascontiguousarray(i_features, np.float32).reshape(16, C, HW)
    pm = np.ascontiguousarray(p_motions, np.float32).reshape(48, 2, 256, 256)

    np_fp8 = mybir.dt.np(FP8)
    np_bf16 = mybir.dt.np(BF16)

    # fkT[g, p, ch, c] = fp8(F[c, q]),  q = 128*ch + p
    f8 = i_features.astype(np_fp8)                 # [16, C, HW]
    fkt_all = np.ascontiguousarray(
        f8.transpose(0, 2, 1).reshape(16, 32, 128, C).transpose(0, 2, 1, 3))

    # ptd[f, i, comp, frow, fcol, j] = pm[f, comp, 4i+1+frow, 4j+1+fcol]
    pmv = pm.reshape(48, 2, 64, 4, 64, 4)[:, :, :, 1:3, :, 1:3]
    ptd_all = np.ascontiguousarray(
        pmv.transpose(0, 2, 1, 3, 5, 4).astype(np.float16))

    # iotau: [:, 0:64, :] = u - p%64 ; [:, 64:76, :] = j - 5 - p//64
    p_idx = np.arange(128)
    iotu = (np.arange(64)[None, :, None] - (p_idx % 64)[:, None, None]
            ).astype(np.float16) + np.zeros((1, 1, 32), np.float16)
    iotw = (np.arange(TW)[None, :, None] - 5 - (p_idx // 64)[:, None, None]
            ).astype(np.float16) + np.zeros((1, 1, 32), np.float16)
    iotau = np.ascontiguousarray(np.concatenate([iotu, iotw], axis=1))

    # esel[r, par, k] = 0.0625 * (r == 2k + par)
    r = np.arange(64)[:, None, None]
    par = np.arange(2)[None, :, None]
    k = np.arange(32)[None, None, :]
    esel = np.ascontiguousarray(
        (0.0625 * (r == 2 * k + par)).astype(np.float16))

    wc = (np.asarray(W_dc, np.float64) @ np.asarray(W_emb, np.float64)) / HW
    # wct16[p, h, o] = wc[o, 128h + p] / HW
    wct16 = np.ascontiguousarray(
        wc.T.reshape(2, 128, C).transpose(1, 0, 2).astype(np.float16))
    be2 = (np.asarray(W_dc, np.float64) @ np.asarray(b_emb, np.float64)) / HW
    be2p = np.ascontiguousarray(
        np.repeat(be2.astype(np.float32)[None, :], 3, axis=0))
    bdc3 = np.ascontiguousarray(
        np.repeat(np.asarray(b_dc, np.float32)[None, :], 3, axis=0))

    in_maps = []
    for cid in range(NCORES):
        in_maps.append({
            "fkt": np.ascontiguousarray(fkt_all[2 * cid:2 * cid + 2]),
            "ptd": np.ascontiguousarray(ptd_all[6 * cid:6 * cid + 6]),
            "iotau": iotau,
            "esel": esel,
            "wct16": wct16,
            "be2p": be2p,
            "bdc3": bdc3,
        })
    return in_maps


def kernel(imgs, i_features, p_motions, W_emb, b_emb, W_dc, b_dc, _trace=False):
    nc = _get_nc()
    in_maps = make_in_maps(np.asarray(i_features), np.asarray(p_motions),
                           np.asarray(W_emb), np.asarray(b_emb),
                           np.asarray(W_dc), np.asarray(b_dc))
    res = run_bass_kernel_spmd(nc, in_maps, core_ids=list(range(NCORES)),
                               trace=_trace)
    out = np.concatenate([np.asarray(r["out"]) for r in res.results], axis=0)
    out = out.reshape(B, NUM_GOP, GOP - 1, C)
    if _trace:
        return out, res
    return out
